# revision 20
# baseline (speedup 1.0000x reference)
"""HRR binding self-attention kernel for 8 trn2 NeuronCores.

Math: out = irfft(c * rfft(x) * cumsum_s(rfft(x))) @ w_out.T  with c = queries*keyvalues.
Since rfft is linear, cumsum commutes with it: only ONE forward DFT of x is needed;
the causal prefix sum runs in the frequency domain.  Two further fusions:
  * irfft followed by the output Linear is one linear map:  out = qv^T (G @ w_out.T),
    precomputed on host as M (packed-spectrum x model_dims).
  * the real per-frequency filter c is diagonal in the packed spectrum, so it folds
    into M as a row scale:  M_c = diag(c_packed) G w_out^T.
So the device does: DFT (matmul, emitted FREQ-major so the spectrum lands in PSUM
with frequency on partitions), causal prefix sum via the DVE's tensor_tensor_scan
(per-partition recurrence along tokens, carry chained through `initial`), complex
pointwise multiply reading Q directly from PSUM, and ONE output matmul with M_c.
No transpose stage, no PSUM->SBUF spectrum eviction, and only ~24 DMAs/iteration.

Sharding: 8 shards = (batch b in 0..3) x (seq half h in 0..1), 2048 tokens each.
The h=1 shards get the first half's contribution as an initial carry, computed on
host as rfft(x[b, :2048].sum(0)) (O(B*D log D) -- negligible).

Packed real spectrum (2048 rows): rows 0..1024 = Re[0..1024], rows 1025..2047 =
Im[1..1023].  Row 1024 (Nyquist, purely real) rides in the Im-block's first slot
(chunk 8, partition 0); complex multiplies pair chunk i with chunk 8+i on equal
partitions, with a 2-row fixup for the DC/Nyquist slots.

Per-core pipeline over 256-token slabs (all matmuls bf16, fp32 PSUM):
  4 DFT passes per slab, each producing 4 freq-chunks {2p, 2p+1, 2p+8, 2p+9}
  (a Re pair and its Im partner pair) into 2 PSUM banks; scan -> S; 6 DVE ops
  -> qv;  the PREVIOUS slab's output matmul (qv chunk^T @ M_c) is interleaved
  between DFT passes so the PE never idles.
"""

import sys

sys.path.insert(0, "/opt/trn_rl_repo")

import numpy as np
import ml_dtypes

import concourse.bass as bass
import concourse.bacc as bacc
import concourse.mybir as mybir
from concourse.tile import TileContext
from concourse.bass_utils import run_bass_kernel_spmd

BF16 = mybir.dt.bfloat16
F32 = mybir.dt.float32
AF = mybir.ActivationFunctionType
ALU = mybir.AluOpType

P = 128
D = 2048  # model dims
T = 2048  # tokens per shard
ND = D // P  # 16 d-chunks
NPF = 16  # packed-frequency chunks
TS = 256  # tokens per slab
NSLAB = T // TS  # 8
NB = 4  # batch
NS = 4096  # full seq

bf16 = ml_dtypes.bfloat16

_CACHE = {}


def _build_nc(reps: int = 1):
    nc = bacc.Bacc("TRN2", target_bir_lowering=False, debug=False, num_devices=8)
    xS = nc.dram_tensor("xS", [NSLAB, P, ND, TS], BF16, kind="ExternalInput")
    CS = nc.dram_tensor("CS", [P, ND, D], BF16, kind="ExternalInput")
    M = nc.dram_tensor("M", [P, NPF, D], BF16, kind="ExternalInput")
    C0 = nc.dram_tensor("C0", [P, NPF], F32, kind="ExternalInput")
    ZR = nc.dram_tensor("ZR", [P, TS], BF16, kind="ExternalInput")
    out = nc.dram_tensor("out", [T, D], BF16, kind="ExternalOutput")

    with TileContext(nc) as tc:
        with tc.tile_pool(name="misc", bufs=1) as misc:
            c0_sb = misc.tile([P, NPF], F32)
            nc.sync.dma_start(c0_sb[:], C0[:])
            zr_sb = misc.tile([P, TS], BF16)
            nc.sync.dma_start(zr_sb[:], ZR[:])
            # weights stay resident across repeat-loop iterations
            cs_sb = misc.tile([P, ND, D], BF16)
            nc.sync.dma_start(cs_sb[:], CS[:])
            m_sb = misc.tile([P, NPF, D], BF16)
            nc.sync.dma_start(m_sb[:], M[:])

            import contextlib

            loop_ctx = (
                tc.For_i(0, reps, 1) if reps > 1 else contextlib.nullcontext()
            )
            with loop_ctx:
                _body(nc, tc, c0_sb, zr_sb, cs_sb, m_sb, xS, out)
    nc.finalize()
    return nc


def _body(nc, tc, c0_sb, zr_sb, cs_sb, m_sb, xS, out):
    with (
        tc.tile_pool(name="xt", bufs=2) as xpool,
        tc.tile_pool(name="ss", bufs=2) as spool,
        tc.tile_pool(name="tmp", bufs=1) as tpool,
        tc.tile_pool(name="qvp", bufs=2) as qvpool,
        tc.tile_pool(name="osb", bufs=2) as opool,
        tc.tile_pool(name="psA", bufs=4, space="PSUM") as psumA,
        tc.tile_pool(name="psB", bufs=4, space="PSUM") as psumB,
    ):
        def emit_B_gen(qv_s, s):
            """Output matmul for one 256-token slab, yielded stepwise so it
            can be interleaved into PE gaps between the next slab's DFT
            passes.  pf-outer order: each 128-token qv chunk is loaded as
            stationary weight ONCE and streamed against all four 512-wide
            M column blocks (4 PSUM banks accumulate in parallel), instead
            of reloading the weight per block."""
            for tsub in range(2):
                # staged/DMA'd as bf16 (host upcasts): halves output traffic
                ob = opool.tile([P, D], BF16, tag="osb")
                psbs = [
                    psumB.tile([P, 512], F32, tag="psB", name=f"psb{e}")
                    for e in range(4)
                ]
                for pf in range(NPF):
                    for e in range(4):
                        nc.tensor.matmul(
                            psbs[e][:],
                            qv_s[:, pf, tsub * P : (tsub + 1) * P],
                            m_sb[:, pf, e * 512 : (e + 1) * 512],
                            start=(pf == 0),
                            stop=(pf == NPF - 1),
                        )
                        yield
                for e in range(4):
                    if e % 2 == 0:
                        nc.scalar.copy(ob[:, e * 512 : (e + 1) * 512], psbs[e][:])
                    else:
                        nc.vector.tensor_copy(
                            ob[:, e * 512 : (e + 1) * 512], psbs[e][:]
                        )
                    yield
                r0 = s * TS + tsub * P
                nc.sync.dma_start(out[r0 : r0 + P, :], ob[:])
                yield

        def adv(gen, n):
            if gen is None:
                return
            for _ in range(n):
                if next(gen, "done") == "done":
                    return

        S_prev = None
        bgen = None
        for s in range(NSLAB):
            xt = xpool.tile([P, ND, TS], BF16, tag="xt")
            nc.sync.dma_start(xt[:], xS[s])
            S_sb = spool.tile([P, NPF, TS], BF16, tag="S")
            qv = qvpool.tile([P, NPF, TS], BF16, tag="qv")
            for p4 in range(4):
                # pass covers a Re chunk pair and its Im partner pair; each
                # frequency chunk accumulates in its OWN psum bank (an
                # accumulation group's start clears has_written bank-wide,
                # so groups must not share banks)
                fcs = [2 * p4, 2 * p4 + 1, 8 + 2 * p4, 8 + 2 * p4 + 1]
                ps4 = [
                    psumA.tile([P, TS], F32, tag="psA", name=f"ps{p4}_{j}")
                    for j in range(4)
                ]
                for d in range(ND):
                    for j, fc in enumerate(fcs):
                        nc.tensor.matmul(
                            ps4[j][:],
                            cs_sb[:, d, fc * P : (fc + 1) * P],
                            xt[:, d, :],
                            start=(d == 0),
                            stop=(d == ND - 1),
                        )
                # previous slab's output matmul fills the PE pipeline while
                # this pass's spectrum is scanned/multiplied
                adv(bgen, 34)

                for j, fc in enumerate(fcs):
                    init = (
                        c0_sb[:, fc : fc + 1]
                        if s == 0
                        else S_prev[:, fc, TS - 1 : TS]
                    )
                    nc.vector.tensor_tensor_scan(
                        S_sb[:, fc, :],
                        ps4[j][:],
                        zr_sb[:],
                        initial=init,
                        op0=ALU.add,
                        op1=ALU.add,
                    )
                for i in range(2):  # the two complex pairs of this pass
                    fre, fim = 2 * p4 + i, 8 + 2 * p4 + i
                    qre, qim = ps4[i], ps4[2 + i]
                    sre, sim_ = S_sb[:, fre, :], S_sb[:, fim, :]
                    t1 = tpool.tile([P, TS], F32, tag="t1")
                    t2 = tpool.tile([P, TS], F32, tag="t2")
                    nc.vector.tensor_mul(t1[:], qre[:], sre)
                    nc.vector.tensor_mul(t2[:], qim[:], sim_)
                    nc.vector.tensor_sub(qv[:, fre, :], t1[:], t2[:])
                    t3 = tpool.tile([P, TS], F32, tag="t1")
                    t4 = tpool.tile([P, TS], F32, tag="t2")
                    nc.vector.tensor_mul(t3[:], qre[:], sim_)
                    nc.vector.tensor_mul(t4[:], qim[:], sre)
                    nc.vector.tensor_add(qv[:, fim, :], t3[:], t4[:])
                    if p4 == 0 and i == 0:
                        # DC (chunk 0 row 0) and Nyquist (chunk 8 row 0):
                        # purely real
                        nc.vector.tensor_mul(
                            qv[0:1, 0, :], qre[0:1, :], S_sb[0:1, 0, :]
                        )
                        nc.vector.tensor_mul(
                            qv[0:1, 8, :], qim[0:1, :], S_sb[0:1, 8, :]
                        )
            S_prev = S_sb

            # drain the rest of the previous slab's output matmul
            adv(bgen, 200)
            bgen = emit_B_gen(qv, s)

        adv(bgen, 200)


def _chunked(m):
    """[rows, cols] -> [P, rows//P, cols] with row r at [r % P, r // P]."""
    r, c = m.shape
    return np.ascontiguousarray(m.reshape(r // P, P, c).transpose(1, 0, 2))


def _pack_spec(re, im):
    """re[1025], im[1025] -> packed [2048]: re[0..1024] then im[1..1023]."""
    return np.concatenate([re, im[1:1024]])


def _constants():
    if "consts" in _CACHE:
        return _CACHE["consts"]
    d = np.arange(D, dtype=np.float64)
    f = np.arange(D // 2 + 1, dtype=np.float64)
    ang = 2.0 * np.pi / D * np.outer(d, f)  # [D, 1025]
    cos, sin = np.cos(ang), np.sin(ang)
    CSf = np.concatenate([cos, -sin[:, 1:1024]], axis=1)  # [D, D]
    alpha = np.full(1025, 2.0)
    alpha[0] = alpha[1024] = 1.0
    Gf = np.concatenate(
        [(alpha[:, None] * cos.T) / D, (-2.0 * sin[:, 1:1024].T) / D], axis=0
    )  # [D packed, D]
    consts = {
        "CS": _chunked(CSf.astype(np.float32)).astype(bf16),
        "Gf32": Gf.astype(np.float32),
    }
    _CACHE["consts"] = consts
    return consts


def prepare_in_maps(x, queries, keyvalues, w_out):
    x = np.asarray(x, dtype=np.float32)
    queries = np.asarray(queries, dtype=np.float32)
    keyvalues = np.asarray(keyvalues, dtype=np.float32)
    w_out = np.asarray(w_out, dtype=np.float32)
    consts = _constants()

    c = (queries * keyvalues).reshape(-1)  # [1025]
    cpk = _pack_spec(c, c).astype(np.float32)  # [2048]
    # irfft + output Linear + c-filter as ONE matrix: M = diag(c_pk) G w_out^T
    Mfull = (consts["Gf32"] * cpk[:, None]) @ np.ascontiguousarray(w_out.T)
    Mc = _chunked(Mfull).astype(bf16)
    zr = np.zeros((P, TS), bf16)

    in_maps = []
    shards = []
    for b in range(NB):
        for h in range(2):
            shards.append((b, h))
            xs = x[b, h * T : (h + 1) * T]  # [T, D]
            xT3 = _chunked(np.ascontiguousarray(xs.T))  # [P, ND, T]
            xSc = np.ascontiguousarray(
                xT3.reshape(P, ND, NSLAB, TS).transpose(2, 0, 1, 3)
            ).astype(bf16)
            if h == 0:
                c0 = np.zeros((P, NPF), np.float32)
            else:
                F = np.fft.rfft(x[b, :T].sum(axis=0).astype(np.float64))
                c0 = _chunked(
                    _pack_spec(F.real, F.imag).astype(np.float32)[:, None]
                )[:, :, 0]
            in_maps.append(
                {
                    "xS": xSc,
                    "CS": consts["CS"],
                    "M": Mc,
                    "C0": np.ascontiguousarray(c0),
                    "ZR": zr,
                }
            )
    return in_maps, shards


def kernel(x, queries, keyvalues, w_out):
    if "nc" not in _CACHE:
        _CACHE["nc"] = _build_nc()
    nc = _CACHE["nc"]
    in_maps, shards = prepare_in_maps(x, queries, keyvalues, w_out)
    res = run_bass_kernel_spmd(nc, in_maps, core_ids=list(range(8)))
    y = np.empty((NB, NS, D), np.float32)
    for i, (b, h) in enumerate(shards):
        y[b, h * T : (h + 1) * T] = np.asarray(res.results[i]["out"]).astype(
            np.float32
        )
    return y


# revision 22
# speedup vs baseline: 2.6814x; 2.6814x over previous
"""HRR binding self-attention kernel for 8 trn2 NeuronCores.

Math: out = irfft(c * rfft(x) * cumsum_s(rfft(x))) @ w_out.T  with c = queries*keyvalues.
Since rfft is linear, cumsum commutes with it: only ONE forward DFT of x is needed;
the causal prefix sum runs in the frequency domain.  Two further fusions:
  * irfft followed by the output Linear is one linear map:  out = qv^T (G @ w_out.T),
    precomputed on host as M (packed-spectrum x model_dims).
  * the real per-frequency filter c is diagonal in the packed spectrum, so it folds
    into M as a row scale:  M_c = diag(c_packed) G w_out^T.
So the device does: DFT (matmul, emitted FREQ-major so the spectrum lands in PSUM
with frequency on partitions), causal prefix sum via the DVE's tensor_tensor_scan
(per-partition recurrence along tokens, carry chained through `initial`), complex
pointwise multiply reading Q directly from PSUM, and ONE output matmul with M_c.

LDWEIGHTS discipline (the dominant non-roofline cost on this part): every
stationary-weight load serializes ~100ns+, so
  * the DFT processes TWO 256-token slabs per pass ("super-slab"), streaming both
    through each CS column chunk while it is resident in the PE array (2x reuse);
  * the output matmul is pf-outer: each 128-token qv chunk is loaded once and
    streamed against all four 512-wide M blocks into 4 parallel PSUM banks (4x).

Sharding: 8 shards = (batch b in 0..3) x (seq half h in 0..1), 2048 tokens each.
The h=1 shards get the first half's contribution as an initial carry, computed on
host as rfft(x[b, :2048].sum(0)) (O(B*D log D) -- negligible).

Packed real spectrum (2048 rows): rows 0..1024 = Re[0..1024], rows 1025..2047 =
Im[1..1023].  Row 1024 (Nyquist, purely real) rides in the Im-block's first slot
(chunk 8, partition 0); complex multiplies pair chunk i with chunk 8+i on equal
partitions, with a 2-row fixup for the DC/Nyquist slots.

The output is staged in half-row bf16 tiles, double-buffered so the B pipeline
never serializes on an output DMA; f32 upcast happens on host (~0.1% extra error).
"""

import sys

sys.path.insert(0, "/opt/trn_rl_repo")

from collections import deque

import numpy as np
import ml_dtypes

import concourse.bass as bass
import concourse.bacc as bacc
import concourse.mybir as mybir
from concourse.tile import TileContext
from concourse.bass_utils import run_bass_kernel_spmd

BF16 = mybir.dt.bfloat16
F32 = mybir.dt.float32
AF = mybir.ActivationFunctionType
ALU = mybir.AluOpType

P = 128
D = 2048  # model dims
T = 2048  # tokens per shard
ND = D // P  # 16 d-chunks
NPF = 16  # packed-frequency chunks
TS = 256  # tokens per slab
NSLAB = T // TS  # 8
NSS = NSLAB // 2  # 4 super-slabs
NB = 4  # batch
NS = 4096  # full seq

bf16 = ml_dtypes.bfloat16

_CACHE = {}


def _build_nc(reps: int = 1):
    nc = bacc.Bacc("TRN2", target_bir_lowering=False, debug=False, num_devices=8)
    xS = nc.dram_tensor("xS", [NSLAB, P, ND, TS], BF16, kind="ExternalInput")
    CS = nc.dram_tensor("CS", [P, ND, D], BF16, kind="ExternalInput")
    M = nc.dram_tensor("M", [P, NPF, D], BF16, kind="ExternalInput")
    C0 = nc.dram_tensor("C0", [P, NPF], F32, kind="ExternalInput")
    ZR = nc.dram_tensor("ZR", [P, TS], BF16, kind="ExternalInput")
    out = nc.dram_tensor("out", [T, D], BF16, kind="ExternalOutput")

    with TileContext(nc) as tc:
        with tc.tile_pool(name="misc", bufs=1) as misc:
            c0_sb = misc.tile([P, NPF], F32)
            nc.sync.dma_start(c0_sb[:], C0[:])
            zr_sb = misc.tile([P, TS], BF16)
            nc.sync.dma_start(zr_sb[:], ZR[:])
            # weights stay resident across repeat-loop iterations
            cs_sb = misc.tile([P, ND, D], BF16)
            nc.sync.dma_start(cs_sb[:], CS[:])
            m_sb = misc.tile([P, NPF, D], BF16)
            nc.sync.dma_start(m_sb[:], M[:])

            import contextlib

            loop_ctx = (
                tc.For_i(0, reps, 1) if reps > 1 else contextlib.nullcontext()
            )
            with loop_ctx:
                _body(nc, tc, c0_sb, zr_sb, cs_sb, m_sb, xS, out)
    nc.finalize()
    return nc


def _body(nc, tc, c0_sb, zr_sb, cs_sb, m_sb, xS, out):
    with (
        tc.tile_pool(name="xt", bufs=3) as xpool,
        tc.tile_pool(name="ss", bufs=2) as spool,
        tc.tile_pool(name="cr", bufs=2) as crpool,
        tc.tile_pool(name="tmp", bufs=1) as tpool,
        tc.tile_pool(name="qvp", bufs=4) as qvpool,
        tc.tile_pool(name="osb", bufs=2) as opool,
        tc.tile_pool(name="psA", bufs=4, space="PSUM") as psumA,
        tc.tile_pool(name="psB", bufs=4, space="PSUM") as psumB,
    ):
        def emit_B_gen(qv_s, s):
            """Output matmul for one 256-token slab, yielded stepwise so it
            can be interleaved into PE gaps between DFT passes.  pf-outer:
            each 128-token qv chunk is loaded as stationary weight ONCE and
            streamed against all four 512-wide M blocks (4 PSUM banks
            accumulate in parallel).  Output staged in double-buffered
            half-row bf16 tiles so evictions never wait on a DMA."""
            for tsub in range(2):
                psbs = [
                    psumB.tile([P, 512], F32, tag="psB", name=f"psb{e}")
                    for e in range(4)
                ]
                for pf in range(NPF):
                    for e in range(4):
                        nc.tensor.matmul(
                            psbs[e][:],
                            qv_s[:, pf, tsub * P : (tsub + 1) * P],
                            m_sb[:, pf, e * 512 : (e + 1) * 512],
                            start=(pf == 0),
                            stop=(pf == NPF - 1),
                        )
                        yield
                r0 = s * TS + tsub * P
                for half in range(2):
                    ob = opool.tile([P, 2, 512], BF16, tag="osb")
                    for k in range(2):
                        e = 2 * half + k
                        if e % 2 == 0:
                            nc.scalar.copy(ob[:, k, :], psbs[e][:])
                        else:
                            nc.vector.tensor_copy(ob[:, k, :], psbs[e][:])
                    nc.sync.dma_start(
                        out[r0 : r0 + P, half * 1024 : (half + 1) * 1024],
                        ob[:],
                    )
                    yield

        bq = deque()

        def adv(n):
            for _ in range(n):
                while bq and next(bq[0], "done") == "done":
                    bq.popleft()
                if not bq:
                    return

        carry_prev = None
        for ss in range(NSS):
            sa, sb_ = 2 * ss, 2 * ss + 1
            xt_a = xpool.tile([P, ND, TS], BF16, tag="xt", name="xt_a")
            nc.sync.dma_start(xt_a[:], xS[sa])
            xt_b = xpool.tile([P, ND, TS], BF16, tag="xt", name="xt_b")
            nc.sync.dma_start(xt_b[:], xS[sb_])
            S_a = spool.tile([P, NPF, TS], BF16, tag="S", name="S_a")
            S_b = spool.tile([P, NPF, TS], BF16, tag="S", name="S_b")
            qv_a = qvpool.tile([P, NPF, TS], BF16, tag="qv", name="qv_a")
            qv_b = qvpool.tile([P, NPF, TS], BF16, tag="qv", name="qv_b")

            for c in range(8):  # one Re/Im chunk pair per pass, both slabs
                ci = c + 8
                pA = psumA.tile([P, TS], F32, tag="psA", name="pA")
                pB = psumA.tile([P, TS], F32, tag="psA", name="pB")
                pA8 = psumA.tile([P, TS], F32, tag="psA", name="pA8")
                pB8 = psumA.tile([P, TS], F32, tag="psA", name="pB8")
                for d in range(ND):
                    # each CS chunk stays resident for both slabs (2x LDW reuse)
                    nc.tensor.matmul(
                        pA[:], cs_sb[:, d, c * P : (c + 1) * P], xt_a[:, d, :],
                        start=(d == 0), stop=(d == ND - 1),
                    )
                    nc.tensor.matmul(
                        pB[:], cs_sb[:, d, c * P : (c + 1) * P], xt_b[:, d, :],
                        start=(d == 0), stop=(d == ND - 1),
                    )
                    nc.tensor.matmul(
                        pA8[:], cs_sb[:, d, ci * P : (ci + 1) * P], xt_a[:, d, :],
                        start=(d == 0), stop=(d == ND - 1),
                    )
                    nc.tensor.matmul(
                        pB8[:], cs_sb[:, d, ci * P : (ci + 1) * P], xt_b[:, d, :],
                        start=(d == 0), stop=(d == ND - 1),
                    )
                adv(35)

                for slab_i, (S_x, qv_x, px, px8) in enumerate(
                    [(S_a, qv_a, pA, pA8), (S_b, qv_b, pB, pB8)]
                ):
                    if slab_i == 0:
                        init_re = (
                            c0_sb[:, c : c + 1]
                            if ss == 0
                            else carry_prev[:, c, 0:1]
                        )
                        init_im = (
                            c0_sb[:, ci : ci + 1]
                            if ss == 0
                            else carry_prev[:, ci, 0:1]
                        )
                    else:
                        init_re = S_a[:, c, TS - 1 : TS]
                        init_im = S_a[:, ci, TS - 1 : TS]
                    nc.vector.tensor_tensor_scan(
                        S_x[:, c, :], px[:], zr_sb[:],
                        initial=init_re, op0=ALU.add, op1=ALU.add,
                    )
                    nc.vector.tensor_tensor_scan(
                        S_x[:, ci, :], px8[:], zr_sb[:],
                        initial=init_im, op0=ALU.add, op1=ALU.add,
                    )
                    sre, sim_ = S_x[:, c, :], S_x[:, ci, :]
                    t1 = tpool.tile([P, TS], BF16, tag="t1")
                    t2 = tpool.tile([P, TS], BF16, tag="t2")
                    nc.vector.tensor_mul(t1[:], px[:], sre)
                    nc.vector.tensor_mul(t2[:], px8[:], sim_)
                    nc.vector.tensor_sub(qv_x[:, c, :], t1[:], t2[:])
                    t3 = tpool.tile([P, TS], BF16, tag="t1")
                    t4 = tpool.tile([P, TS], BF16, tag="t2")
                    nc.vector.tensor_mul(t3[:], px[:], sim_)
                    nc.vector.tensor_mul(t4[:], px8[:], sre)
                    nc.vector.tensor_add(qv_x[:, ci, :], t3[:], t4[:])
                    if c == 0:
                        # DC (chunk 0 row 0) / Nyquist (chunk 8 row 0): real
                        nc.vector.tensor_mul(
                            qv_x[0:1, 0, :], px[0:1, :], S_x[0:1, 0, :]
                        )
                        nc.vector.tensor_mul(
                            qv_x[0:1, 8, :], px8[0:1, :], S_x[0:1, 8, :]
                        )

            # snapshot the carry columns so the S slots free at super-slab end
            # (slab-a scans of the NEXT super-slab read only this tiny tile)
            carry_prev = crpool.tile([P, NPF, 1], BF16, tag="carry")
            nc.vector.tensor_copy(carry_prev[:], S_b[:, :, TS - 1 : TS])

            adv(300)  # drain any remaining queued output matmul
            bq.append(emit_B_gen(qv_a, sa))
            bq.append(emit_B_gen(qv_b, sb_))

        adv(300)


def _chunked(m):
    """[rows, cols] -> [P, rows//P, cols] with row r at [r % P, r // P]."""
    r, c = m.shape
    return np.ascontiguousarray(m.reshape(r // P, P, c).transpose(1, 0, 2))


def _pack_spec(re, im):
    """re[1025], im[1025] -> packed [2048]: re[0..1024] then im[1..1023]."""
    return np.concatenate([re, im[1:1024]])


def _constants():
    if "consts" in _CACHE:
        return _CACHE["consts"]
    d = np.arange(D, dtype=np.float64)
    f = np.arange(D // 2 + 1, dtype=np.float64)
    ang = 2.0 * np.pi / D * np.outer(d, f)  # [D, 1025]
    cos, sin = np.cos(ang), np.sin(ang)
    CSf = np.concatenate([cos, -sin[:, 1:1024]], axis=1)  # [D, D]
    alpha = np.full(1025, 2.0)
    alpha[0] = alpha[1024] = 1.0
    Gf = np.concatenate(
        [(alpha[:, None] * cos.T) / D, (-2.0 * sin[:, 1:1024].T) / D], axis=0
    )  # [D packed, D]
    consts = {
        "CS": _chunked(CSf.astype(np.float32)).astype(bf16),
        "Gf32": Gf.astype(np.float32),
    }
    _CACHE["consts"] = consts
    return consts


def prepare_in_maps(x, queries, keyvalues, w_out):
    x = np.asarray(x, dtype=np.float32)
    queries = np.asarray(queries, dtype=np.float32)
    keyvalues = np.asarray(keyvalues, dtype=np.float32)
    w_out = np.asarray(w_out, dtype=np.float32)
    consts = _constants()

    c = (queries * keyvalues).reshape(-1)  # [1025]
    cpk = _pack_spec(c, c).astype(np.float32)  # [2048]
    # irfft + output Linear + c-filter as ONE matrix: M = diag(c_pk) G w_out^T
    Mfull = (consts["Gf32"] * cpk[:, None]) @ np.ascontiguousarray(w_out.T)
    Mc = _chunked(Mfull).astype(bf16)
    zr = np.zeros((P, TS), bf16)

    in_maps = []
    shards = []
    for b in range(NB):
        for h in range(2):
            shards.append((b, h))
            xs = x[b, h * T : (h + 1) * T]  # [T, D]
            xT3 = _chunked(np.ascontiguousarray(xs.T))  # [P, ND, T]
            xSc = np.ascontiguousarray(
                xT3.reshape(P, ND, NSLAB, TS).transpose(2, 0, 1, 3)
            ).astype(bf16)
            if h == 0:
                c0 = np.zeros((P, NPF), np.float32)
            else:
                F = np.fft.rfft(x[b, :T].sum(axis=0).astype(np.float64))
                c0 = _chunked(
                    _pack_spec(F.real, F.imag).astype(np.float32)[:, None]
                )[:, :, 0]
            in_maps.append(
                {
                    "xS": xSc,
                    "CS": consts["CS"],
                    "M": Mc,
                    "C0": np.ascontiguousarray(c0),
                    "ZR": zr,
                }
            )
    return in_maps, shards


def kernel(x, queries, keyvalues, w_out):
    if "nc" not in _CACHE:
        _CACHE["nc"] = _build_nc()
    nc = _CACHE["nc"]
    in_maps, shards = prepare_in_maps(x, queries, keyvalues, w_out)
    res = run_bass_kernel_spmd(nc, in_maps, core_ids=list(range(8)))
    y = np.empty((NB, NS, D), np.float32)
    for i, (b, h) in enumerate(shards):
        y[b, h * T : (h + 1) * T] = np.asarray(res.results[i]["out"]).astype(
            np.float32
        )
    return y
